# revision 1
# baseline (speedup 1.0000x reference)
"""Trainium2 Bass kernel for nn_MaxMinAgg.

Computes, for full inputs m [1024, 256] f32 and weight [256, 512] f32:
    z[b, j]  = max_k min(m[b, k], weight[k, j])          (tropical max-min matmul)
    out[b,o] = max_a z[b, 4*o + a]                       (max-pool over AGG=4 groups)

Key identity: max_a min(x, w_a) = min(x, max_a w_a): the AGG max-pool folds into
the weight (wmax[k, o] = max_a weight[k, 4o+a]), 4x less elementwise work, and
    out[b, o] = max_k min(m[b, k], wmax[k, o])
All ops are exact f32 selections -> bit-exact result.

Distribution: data-parallel over batch across 8 NeuronCores (128 rows each);
weight replicated.

Per-core algorithm. The elementwise min+max-reduce streams ~2 passes over
b*o*k/core on the DVE (the only engine with a 2-tensor min) - that is the time
floor; everything else hides under/around it:
  - Partitions carry p = kg*64 + og (kg in {0,1} k-halves, og in [0,64) output
    groups): partition p handles outputs o = t*64+og (2 o-blocks) and k-half
    [kg*128, kg*128+128).  m is DMA-broadcast from DRAM with only 64x
    replication (8MB) in 512B-contiguous runs, b-chunked so compute starts
    while m still streams.
  - Weight: one segmented reduce folds AGG -> wmax; two PE transposes ->
    wmaxT [o, k]; wmaxT round-trips through DRAM so per-o-block weight tiles
    wblock[p, k'] land in the partition layout (transpose outputs must start
    at PSUM partition 0, so direct placement is impossible).
  - Per o-block t: DVE tensor_tensor min (wblock free-broadcast over b vs
    mrep) + segmented tensor_reduce max over the k-half -> partial[p, b];
    PE-transpose partial and a tiny strided DVE max-reduce over the 2 kg
    slots emits out[b, t-block] in natural layout (no final transpose).
"""

import sys

import numpy as np

if "/opt/trn_rl_repo" not in sys.path:
    sys.path.insert(0, "/opt/trn_rl_repo")

B, IN_F, OUT_F, AGG = 1024, 256, 128, 4
N_CORES = 8
B_SH = B // N_CORES  # 128

KG, OG = 2, 64  # partition factorization: p = kg*OG + og
KS = IN_F // KG  # 128 k per group
NT = OUT_F // OG  # 2 o-blocks

# b-chunks (compute starts while m still streams in).
B_CHUNKS = [16, 32, 80]

_CACHE = {}


def emit_core_program(tc, o_d, m_d, w_d):
    """Emit the per-core Tile program.

    o_d: DRAM out [B_SH, OUT_F] f32, m_d: DRAM in [B_SH, IN_F] f32,
    w_d: DRAM in [IN_F, OUT_F*AGG] f32.
    """
    from contextlib import ExitStack

    import concourse.bass as bass
    from concourse import mybir
    from concourse.masks import make_identity

    nc = tc.nc
    f32 = mybir.dt.float32
    AX = mybir.AxisListType
    OP = mybir.AluOpType

    with ExitStack() as ctx:
        const = ctx.enter_context(tc.tile_pool(name="const", bufs=1))
        mintp = ctx.enter_context(tc.tile_pool(name="mintp", bufs=2))
        partp = ctx.enter_context(tc.tile_pool(name="partp", bufs=2))
        ps_tr = ctx.enter_context(tc.tile_pool(name="ps_tr", bufs=2, space="PSUM"))

        # --- weight load first (scalar queue, ahead of the bulk) -----------
        w_sb = const.tile([128, 2, OUT_F * AGG], f32)
        wv = w_d.rearrange("(h p) j -> p h j", p=128)
        nc.scalar.dma_start(out=w_sb[:, 0, :], in_=wv[:, 0, :])
        nc.scalar.dma_start(out=w_sb[:, 1, :], in_=wv[:, 1, :])

        # --- m broadcast: partition p = kg*OG+og gets m[b, kg*KS:(kg+1)*KS],
        # replicated over the 64 og's (8MB total, 512B contiguous runs).
        # One tile per b-chunk so compute unblocks per chunk.  All bulk rides
        # the scalar queue (the sync queue measures ~3x slower); the tiny
        # weight-side transfers ride sync so they never sit behind the bulk.
        mreps = []

        def emit_mrep_chunk(ci, b0, bc):
            mrep = const.tile([128, bc, KS], f32, name=f"mrep{ci}", uniquify=True)
            for kg in range(KG):
                src = bass.AP(
                    tensor=m_d.tensor,
                    offset=m_d.offset + b0 * IN_F + kg * KS,
                    ap=[[0, OG], [IN_F, bc], [1, KS]],
                )
                nc.scalar.dma_start(
                    out=mrep[kg * OG : (kg + 1) * OG, :, :], in_=src
                )
            mreps.append(mrep)

        emit_mrep_chunk(0, 0, B_CHUNKS[0])

        # --- weight fold: wmax[k_p, h, o] = max_a w[k, 4o+a] ---------------
        wmax_sb = const.tile([128, 2, OUT_F], f32)
        nc.vector.tensor_reduce(
            out=wmax_sb,
            in_=w_sb.rearrange("p h (o a) -> p h o a", a=AGG),
            axis=AX.X,
            op=OP.max,
        )

        ident = const.tile([128, 128], f32)
        make_identity(nc, ident)

        # wmaxT [o, k] via two PE transposes, then to DRAM so the per-block
        # weight tiles can be fetched in the p = kg*OG+og partition layout
        # (transpose outputs must land at PSUM partition 0, so direct
        # placement at partition offsets is impossible).
        wmaxT = const.tile([128, 2, 128], f32)
        for h in range(2):
            pt = ps_tr.tile([128, 128], f32, tag="ptr")
            nc.tensor.transpose(pt, wmax_sb[:, h, :], ident)
            nc.vector.tensor_copy(wmaxT[:, h, :], pt)
        wT_d = nc.dram_tensor("wT_scratch", [OUT_F, IN_F], f32, kind="Internal").ap()
        nc.scalar.dma_start(out=wT_d, in_=wmaxT)

        # wblock_t[p=kg*OG+og, k'] = wmaxT[t*OG+og, kg*KS+k']
        wbs = []
        for t in range(NT):
            wb = const.tile([128, KS], f32, tag="wb", bufs=2, name=f"wb{t}")
            src = bass.AP(
                tensor=wT_d.tensor,
                offset=wT_d.offset + t * OG * IN_F,
                ap=[[KS, KG], [IN_F, OG], [1, KS]],
            )
            nc.scalar.dma_start(out=wb, in_=src)
            wbs.append(wb)

        # remaining m chunks, behind the (tiny) weight-chain transfers
        b0 = B_CHUNKS[0]
        for ci, bc in enumerate(B_CHUNKS[1:], start=1):
            emit_mrep_chunk(ci, b0, bc)
            b0 += bc

        out_sb = const.tile([B_SH, OUT_F], f32)
        partials = [
            const.tile([128, B_SH], f32, name=f"partial{t}") for t in range(NT)
        ]

        # chunk-major: each m chunk is consumed for both o-blocks as soon as
        # it lands; DVE stays dense while later chunks stream in.
        b0 = 0
        for ci, bc in enumerate(B_CHUNKS):
            for t in range(NT):
                mint = mintp.tile([128, max(B_CHUNKS), KS], f32, tag="mint")
                nc.vector.tensor_tensor(
                    out=mint[:, :bc, :],
                    in0=wbs[t]
                    .rearrange("p k -> p () k")
                    .broadcast_to((128, bc, KS)),
                    in1=mreps[ci],
                    op=OP.min,
                )
                nc.vector.tensor_reduce(
                    out=partials[t][:, b0 : b0 + bc],
                    in_=mint[:, :bc, :],
                    axis=AX.X,
                    op=OP.max,
                )
            b0 += bc

        # transpose partial [p, b] -> [b, p], combine the KG kg-slots
        for t in range(NT):
            ptr = ps_tr.tile([128, 128], f32, tag="ptr")
            nc.tensor.transpose(ptr, partials[t], ident)
            nc.vector.tensor_reduce(
                out=out_sb[:, t * OG : (t + 1) * OG],
                in_=ptr.rearrange("b (kg og) -> b og kg", kg=KG),
                axis=AX.X,
                op=OP.max,
            )

        nc.sync.dma_start(out=o_d, in_=out_sb)


def _build():
    if "nc" in _CACHE:
        return _CACHE["nc"]
    import concourse.bacc as bacc
    import concourse.tile as tile
    from concourse import mybir

    f32 = mybir.dt.float32
    nc = bacc.Bacc(
        "TRN2",
        target_bir_lowering=False,
        debug=False,
        enable_asserts=True,
        num_devices=N_CORES,
    )
    m_d = nc.dram_tensor("m0", [B_SH, IN_F], f32, kind="ExternalInput").ap()
    w_d = nc.dram_tensor("w0", [IN_F, OUT_F * AGG], f32, kind="ExternalInput").ap()
    o_d = nc.dram_tensor("out0", [B_SH, OUT_F], f32, kind="ExternalOutput").ap()
    with tile.TileContext(nc) as tc:
        emit_core_program(tc, o_d, m_d, w_d)
    nc.compile()
    _CACHE["nc"] = nc
    return nc


def run(m, weight, trace=False, **spmd_kwargs):
    """Run on 8 NeuronCores; returns (full_output, BassKernelResults)."""
    from concourse.bass_utils import run_bass_kernel_spmd

    nc = _build()
    m = np.ascontiguousarray(np.asarray(m, dtype=np.float32))
    weight = np.ascontiguousarray(np.asarray(weight, dtype=np.float32))
    assert m.shape == (B, IN_F) and weight.shape == (IN_F, OUT_F * AGG)
    in_maps = [
        {"m0": m[i * B_SH : (i + 1) * B_SH], "w0": weight} for i in range(N_CORES)
    ]
    res = run_bass_kernel_spmd(
        nc, in_maps, core_ids=list(range(N_CORES)), trace=trace, **spmd_kwargs
    )
    out = np.concatenate([res.results[i]["out0"] for i in range(N_CORES)], axis=0)
    return out, res


def kernel(m, weight, agg_features=AGG, **_ignored):
    assert int(agg_features) == AGG
    out, _ = run(m, weight, trace=False)
    return out.astype(np.float32)



# revision 3
# speedup vs baseline: 1.5319x; 1.5319x over previous
"""Trainium2 Bass kernel for nn_MaxMinAgg.

Computes, for full inputs m [1024, 256] f32 and weight [256, 512] f32:
    z[b, j]  = max_k min(m[b, k], weight[k, j])          (tropical max-min matmul)
    out[b,o] = max_a z[b, 4*o + a]                       (max-pool over AGG=4 groups)

Identity: max_a min(x, w_a) = min(x, max_a w_a): the AGG max-pool folds into the
weight (wmax[k, o] = max_a weight[k, 4o+a]), so
    out[b, o] = max_k min(m[b, k], wmax[k, o])

Distribution: data-parallel over batch across 8 NeuronCores (128 rows each);
weight replicated.

Per-core algorithm (v2 — bf16 + low-replication layout + max-tree):
  - All compute in bf16: min/max are exact selections, so the only error is the
    initial f32->bf16 rounding (<= 2^-9 relative, far under the 2e-2 gate), and
    bf16 unlocks the DVE's 2x packed mode for tensor_tensor (tensor_reduce is
    always 1x, hence the halving max-tree below).
  - Partition layout p = og*16 + bg (OG=8 output groups x BG=16 batch groups).
    Partition p owns the full k-reduction for an 8b' x 16o' output tile, so
    k (=256) lives on the free axis and there is NO cross-partition combine:
    no epilogue transposes, the final reduce lands in natural layout.
  - Replication traffic: w_rep = wmaxT broadcast over the 16 bg (1MB bf16) and
    m broadcast over the 8 og (1MB f32, read straight from the input DRAM with
    8KB-contiguous runs), vs 8MB in the v1 layout. m is converted to bf16
    on-chip by the ACT engine (DVE untouched).
  - Weight prep: DVE segmented tensor_reduce folds AGG; two PE transposes give
    wmaxT [o, k]; wmaxT round-trips through DRAM so the per-partition slabs
    [o', k] can be fetched with plain contiguous broadcast reads.
  - Hot loop per o'-half: one bf16 tensor_tensor min (m free-broadcast over o',
    w_rep free-broadcast over b') then a halving tensor_tensor max tree over k
    (256->128->64->32->16, all 2x mode, contiguous halves so packing holds) and
    one small 1x tensor_reduce for the last 16. ACT converts the bf16 partials
    to f32 and the out DMA scatters them to the natural [b, o] layout.
"""

import sys

import numpy as np

if "/opt/trn_rl_repo" not in sys.path:
    sys.path.insert(0, "/opt/trn_rl_repo")

B, IN_F, OUT_F, AGG = 1024, 256, 128, 4
N_CORES = 8
B_SH = B // N_CORES  # 128

OG, BG = 8, 16  # partition p = og*BG + bg
BP = B_SH // BG  # 8 batch rows per partition
OPP = OUT_F // OG  # 16 output cols per partition
NT = 2  # o'-halves (chunks of the hot loop)
OC = OPP // NT  # 8 output cols per chunk

_CACHE = {}


def emit_core_program(tc, o_d, m_d, w_d):
    """Emit the per-core Tile program.

    o_d: DRAM out [B_SH, OUT_F] f32, m_d: DRAM in [B_SH, IN_F] f32,
    w_d: DRAM in [IN_F, OUT_F*AGG] f32.
    """
    from contextlib import ExitStack

    import concourse.bass as bass
    from concourse import mybir
    from concourse.masks import make_identity

    nc = tc.nc
    f32 = mybir.dt.float32
    bf16 = mybir.dt.bfloat16
    AX = mybir.AxisListType
    OP = mybir.AluOpType

    with ExitStack() as ctx:
        const = ctx.enter_context(tc.tile_pool(name="const", bufs=1))
        mintp = ctx.enter_context(tc.tile_pool(name="mintp", bufs=2))
        treep = ctx.enter_context(tc.tile_pool(name="treep", bufs=2))
        ps_tr = ctx.enter_context(tc.tile_pool(name="ps_tr", bufs=2, space="PSUM"))

        # --- weight load [k_p, h, j] f32 (scalar queue, heads the w chain) --
        w_sb = const.tile([128, 2, OUT_F * AGG], f32)
        wv = w_d.rearrange("(h p) j -> p h j", p=128)
        nc.scalar.dma_start(out=w_sb[:, 0, :], in_=wv[:, 0, :])
        nc.scalar.dma_start(out=w_sb[:, 1, :], in_=wv[:, 1, :])

        # --- m broadcast-read: partition p = og*BG+bg gets m[bg*BP:(bg+1)*BP, :]
        # replicated over the 8 og's. 1MB f32 in 8KB-contiguous runs, on the
        # gpsimd queue so it streams in parallel with the scalar-queue w chain.
        m32 = const.tile([128, BP * IN_F], f32)
        m_src = bass.AP(
            tensor=m_d.tensor,
            offset=m_d.offset,
            ap=[[0, OG], [BP * IN_F, BG], [1, BP * IN_F]],
        )
        ident = const.tile([128, 128], bf16)
        make_identity(nc, ident)  # gpsimd compute, before the bulk DMA issue
        nc.gpsimd.dma_start(out=m32, in_=m_src)

        # --- weight fold: wmax[k_p, h, o] = max_a w[k, 4o+a], f32 -> bf16 ----
        wmax16 = const.tile([128, 2, OUT_F], bf16)
        nc.vector.tensor_reduce(
            out=wmax16,
            in_=w_sb.rearrange("p h (o a) -> p h o a", a=AGG),
            axis=AX.X,
            op=OP.max,
        )

        # wmaxT [o, k] via two PE transposes; ACT copies PSUM f32 -> bf16 SBUF.
        wmaxT16 = const.tile([128, 2, 128], bf16)  # [o_p, h, k'] == [o, k]
        for h in range(2):
            pt = ps_tr.tile([128, 128], bf16, tag="ptr")
            nc.tensor.transpose(pt, wmax16[:, h, :], ident)
            nc.scalar.copy(out=wmaxT16[:, h, :], in_=pt)

        # wmaxT -> DRAM so the per-partition slabs [o', k] land via plain
        # contiguous broadcast reads (transpose outputs must start at PSUM
        # partition 0, so direct placement at partition offsets is impossible).
        wT_d = nc.dram_tensor("wT_scratch", [OUT_F, IN_F], bf16, kind="Internal").ap()
        nc.scalar.dma_start(out=wT_d, in_=wmaxT16)

        # w_rep[t][p = og*BG+bg, o', k] = wmaxT[og*OPP + t*OC + o', k]
        wreps = []
        for t in range(NT):
            wr = const.tile([128, OC, IN_F], bf16, name=f"wrep{t}")
            src = bass.AP(
                tensor=wT_d.tensor,
                offset=wT_d.offset + t * OC * IN_F,
                ap=[[OPP * IN_F, OG], [0, BG], [1, OC * IN_F]],
            )
            nc.scalar.dma_start(out=wr, in_=src)
            wreps.append(wr)

        # m f32 -> bf16 on ACT (emitted after the w-chain DMAs so the convert
        # never delays their issue on the shared scalar queue).
        m16 = const.tile([128, BP * IN_F], bf16)
        half = BP * IN_F // 2
        for i in range(2):
            nc.scalar.copy(
                out=m16[:, i * half : (i + 1) * half],
                in_=m32[:, i * half : (i + 1) * half],
            )
        m16v = m16.rearrange("p (b k) -> p b () k", k=IN_F)

        # --- hot loop: per o'-half, bf16 min + halving max-tree over k ------
        for t in range(NT):
            mint = mintp.tile([128, BP, OC, IN_F], bf16, tag="mint")
            nc.vector.tensor_tensor(
                out=mint,
                in0=m16v.broadcast_to((128, BP, OC, IN_F)),
                in1=wreps[t]
                .rearrange("p o k -> p () o k")
                .broadcast_to((128, BP, OC, IN_F)),
                op=OP.min,
            )
            cur, n = mint, IN_F
            while n > 16:
                h = n // 2
                nxt = treep.tile([128, BP, OC, h], bf16, tag=f"tree{h}")
                nc.vector.tensor_tensor(
                    out=nxt,
                    in0=cur[:, :, :, :h],
                    in1=cur[:, :, :, h:],
                    op=OP.max,
                )
                cur, n = nxt, h
            part = treep.tile([128, BP, OC], bf16, tag="part")
            nc.vector.tensor_reduce(out=part, in_=cur, axis=AX.X, op=OP.max)
            outc = treep.tile([128, BP, OC], f32, tag="outc")
            nc.scalar.copy(out=outc, in_=part)
            dst = bass.AP(
                tensor=o_d.tensor,
                offset=o_d.offset + t * OC,
                ap=[[OPP, OG], [BP * OUT_F, BG], [OUT_F, BP], [1, OC]],
            )
            nc.sync.dma_start(out=dst, in_=outc)


def _build():
    if "nc" in _CACHE:
        return _CACHE["nc"]
    import concourse.bacc as bacc
    import concourse.tile as tile
    from concourse import mybir

    f32 = mybir.dt.float32
    nc = bacc.Bacc(
        "TRN2",
        target_bir_lowering=False,
        debug=False,
        enable_asserts=True,
        num_devices=N_CORES,
    )
    m_d = nc.dram_tensor("m0", [B_SH, IN_F], f32, kind="ExternalInput").ap()
    w_d = nc.dram_tensor("w0", [IN_F, OUT_F * AGG], f32, kind="ExternalInput").ap()
    o_d = nc.dram_tensor("out0", [B_SH, OUT_F], f32, kind="ExternalOutput").ap()
    with tile.TileContext(nc) as tc:
        emit_core_program(tc, o_d, m_d, w_d)
    nc.compile()
    _CACHE["nc"] = nc
    return nc


def run(m, weight, trace=False, **spmd_kwargs):
    """Run on 8 NeuronCores; returns (full_output, BassKernelResults)."""
    from concourse.bass_utils import run_bass_kernel_spmd

    nc = _build()
    m = np.ascontiguousarray(np.asarray(m, dtype=np.float32))
    weight = np.ascontiguousarray(np.asarray(weight, dtype=np.float32))
    assert m.shape == (B, IN_F) and weight.shape == (IN_F, OUT_F * AGG)
    in_maps = [
        {"m0": m[i * B_SH : (i + 1) * B_SH], "w0": weight} for i in range(N_CORES)
    ]
    res = run_bass_kernel_spmd(
        nc, in_maps, core_ids=list(range(N_CORES)), trace=trace, **spmd_kwargs
    )
    out = np.concatenate([res.results[i]["out0"] for i in range(N_CORES)], axis=0)
    return out, res


def kernel(m, weight, agg_features=AGG, **_ignored):
    assert int(agg_features) == AGG
    out, _ = run(m, weight, trace=False)
    return out.astype(np.float32)


# revision 4
# speedup vs baseline: 1.5922x; 1.0393x over previous
"""Trainium2 Bass kernel for nn_MaxMinAgg.

Computes, for full inputs m [1024, 256] f32 and weight [256, 512] f32:
    z[b, j]  = max_k min(m[b, k], weight[k, j])          (tropical max-min matmul)
    out[b,o] = max_a z[b, 4*o + a]                       (max-pool over AGG=4 groups)

Identity: max_a min(x, w_a) = min(x, max_a w_a): the AGG max-pool folds into the
weight (wmax[k, o] = max_a weight[k, 4o+a]), so
    out[b, o] = max_k min(m[b, k], wmax[k, o])

Distribution: data-parallel over batch across 8 NeuronCores (128 rows each);
weight replicated.

Per-core algorithm (v3 — bf16, low-replication layout, max-tree, 2-queue DMA):
  - All compute in bf16: min/max are exact selections, so the only error is the
    initial f32->bf16 rounding (<= 2^-9 relative, far under the 2e-2 gate), and
    bf16 unlocks the DVE's 2x packed mode for tensor_tensor (tensor_reduce is
    always 1x, hence the halving max-tree below).
  - Partition layout p = og*16 + bg (OG=8 output groups x BG=16 batch groups).
    Partition p owns the full k-reduction for an 8b' x 16o' output tile, so
    k (=256) lives on the free axis and there is NO cross-partition combine:
    no epilogue transposes, the final reduce lands in natural layout.
  - Replication: w_rep = wmaxT broadcast over the 16 bg (1MB bf16), m broadcast
    over the 8 og (1MB f32 read straight from the input with 8KB runs). ACT
    converts m to bf16 on-chip; the PE transposes wmax; DVE only ever runs the
    fold + the hot loop.
  - Only the scalar and sync queues have hardware descriptor generators, and
    each sustains ~140GB/s with 4KB packets, so the bulk is balanced across
    exactly those two: scalar carries the weight chain (w halves split across
    both queues, then wT out / w_rep in), sync carries m32 + the output
    scatter. The identity for the PE transpose is fed from the host as an
    extra input, so the gpsimd engine is not used at all.
  - Hot loop per o'-half: bf16 tensor_tensor min (m free-broadcast over o',
    w_rep free-broadcast over b'; the first chunk's min is split in two b'
    halves so it can start before the second m32 chunk converts), then a
    halving tensor_tensor max tree over k (256->...->16, all 2x mode,
    contiguous halves so packing holds), one small 1x tensor_reduce for the
    last 16, ACT convert to f32, and a scatter-DMA to the natural [b, o]
    layout.
"""

import sys

import numpy as np

if "/opt/trn_rl_repo" not in sys.path:
    sys.path.insert(0, "/opt/trn_rl_repo")

B, IN_F, OUT_F, AGG = 1024, 256, 128, 4
N_CORES = 8
B_SH = B // N_CORES  # 128

OG, BG = 8, 16  # partition p = og*BG + bg
BP = B_SH // BG  # 8 batch rows per partition
OPP = OUT_F // OG  # 16 output cols per partition
NT = 2  # o'-halves (chunks of the hot loop)
OC = OPP // NT  # 8 output cols per chunk

_CACHE = {}


def _identity_np():
    import ml_dtypes

    return np.eye(128, dtype=ml_dtypes.bfloat16)


def emit_core_program(tc, o_d, m_d, w_d, i_d):
    """Emit the per-core Tile program.

    o_d: DRAM out [B_SH, OUT_F] f32, m_d: DRAM in [B_SH, IN_F] f32,
    w_d: DRAM in [IN_F, OUT_F*AGG] f32, i_d: DRAM in [128, 128] bf16 identity.
    """
    from contextlib import ExitStack

    import concourse.bass as bass
    from concourse import mybir

    nc = tc.nc
    f32 = mybir.dt.float32
    bf16 = mybir.dt.bfloat16
    AX = mybir.AxisListType
    OP = mybir.AluOpType

    with ExitStack() as ctx:
        const = ctx.enter_context(tc.tile_pool(name="const", bufs=1))
        mintp = ctx.enter_context(tc.tile_pool(name="mintp", bufs=2))
        treep = ctx.enter_context(tc.tile_pool(name="treep", bufs=2))
        ps_tr = ctx.enter_context(tc.tile_pool(name="ps_tr", bufs=2, space="PSUM"))

        # --- weight load [k_p, h, j] f32, halves split across the two HW
        # descriptor queues (scalar / sync) so both stream in parallel.
        w_sb = const.tile([128, 2, OUT_F * AGG], f32)
        wv = w_d.rearrange("(h p) j -> p h j", p=128)
        nc.scalar.dma_start(out=w_sb[:, 0, :], in_=wv[:, 0, :])
        nc.sync.dma_start(out=w_sb[:, 1, :], in_=wv[:, 1, :])

        ident = const.tile([128, 128], bf16)
        nc.scalar.dma_start(out=ident, in_=i_d)

        # --- m broadcast-read: partition p = og*BG+bg gets m[bg*BP:(bg+1)*BP, :]
        # replicated over the 8 og's. 1MB f32 in 8KB runs, chunked in two b'
        # halves on the sync queue; ACT converts each half to bf16 as it lands.
        m32 = const.tile([128, BP * IN_F], f32)
        HB = BP * IN_F // 2  # elements per b'-half, per partition
        for c in range(2):
            src = bass.AP(
                tensor=m_d.tensor,
                offset=m_d.offset + c * HB,
                ap=[[0, OG], [BP * IN_F, BG], [1, HB]],
            )
            nc.sync.dma_start(out=m32[:, c * HB : (c + 1) * HB], in_=src)

        # --- weight fold + transpose, pipelined per k-half h ----------------
        # wmax[k_p, h, o] = max_a w[k, 4o+a] (f32 -> bf16), then PE transpose
        # and ACT copy PSUM -> wmaxT16 [o_p, h, k'] == wmaxT [o, k] bf16.
        wmax16 = const.tile([128, 2, OUT_F], bf16)
        wmaxT16 = const.tile([128, 2, 128], bf16)
        for h in range(2):
            nc.vector.tensor_reduce(
                out=wmax16[:, h, :],
                in_=w_sb.rearrange("p h (o a) -> p h o a", a=AGG)[:, h, :, :],
                axis=AX.X,
                op=OP.max,
            )
            pt = ps_tr.tile([128, 128], bf16, tag="ptr")
            nc.tensor.transpose(pt, wmax16[:, h, :], ident)
            nc.scalar.copy(out=wmaxT16[:, h, :], in_=pt)

        # wmaxT -> DRAM so the per-partition slabs [o', k] land via plain
        # contiguous broadcast reads (transpose outputs must start at PSUM
        # partition 0, so direct placement at partition offsets is impossible).
        wT_d = nc.dram_tensor("wT_scratch", [OUT_F, IN_F], bf16, kind="Internal").ap()
        nc.scalar.dma_start(out=wT_d, in_=wmaxT16)

        # w_rep[t][p = og*BG+bg, o', k] = wmaxT[og*OPP + t*OC + o', k]
        wreps = []
        for t in range(NT):
            wr = const.tile([128, OC, IN_F], bf16, name=f"wrep{t}")
            src = bass.AP(
                tensor=wT_d.tensor,
                offset=wT_d.offset + t * OC * IN_F,
                ap=[[OPP * IN_F, OG], [0, BG], [1, OC * IN_F]],
            )
            nc.scalar.dma_start(out=wr, in_=src)
            wreps.append(wr)

        # m f32 -> bf16 on ACT, one op per b'-half (emitted after the w-chain
        # DMAs so the converts never delay their issue on the scalar queue).
        m16 = const.tile([128, BP * IN_F], bf16)
        for c in range(2):
            nc.scalar.copy(
                out=m16[:, c * HB : (c + 1) * HB],
                in_=m32[:, c * HB : (c + 1) * HB],
            )
        m16v = m16.rearrange("p (b k) -> p b () k", k=IN_F)

        # --- hot loop: per o'-half, bf16 min + halving max-tree over k ------
        for t in range(NT):
            mint = mintp.tile([128, BP, OC, IN_F], bf16, tag="mint")
            # Chunk 0's min is split in two b' halves: the first half only
            # needs the first m32 chunk, so it starts as soon as w_rep0 lands.
            bsplits = (2, 1)[t]
            bs = BP // bsplits
            for s in range(bsplits):
                nc.vector.tensor_tensor(
                    out=mint[:, s * bs : (s + 1) * bs, :, :],
                    in0=m16v[:, s * bs : (s + 1) * bs, :, :].broadcast_to(
                        (128, bs, OC, IN_F)
                    ),
                    in1=wreps[t]
                    .rearrange("p o k -> p () o k")
                    .broadcast_to((128, bs, OC, IN_F)),
                    op=OP.min,
                )
            cur, n = mint, IN_F
            while n > 16:
                h = n // 2
                nxt = treep.tile([128, BP, OC, h], bf16, tag=f"tree{h}")
                nc.vector.tensor_tensor(
                    out=nxt,
                    in0=cur[:, :, :, :h],
                    in1=cur[:, :, :, h:],
                    op=OP.max,
                )
                cur, n = nxt, h
            part = treep.tile([128, BP, OC], bf16, tag="part")
            nc.vector.tensor_reduce(out=part, in_=cur, axis=AX.X, op=OP.max)
            outc = treep.tile([128, BP, OC], f32, tag="outc")
            nc.scalar.copy(out=outc, in_=part)
            dst = bass.AP(
                tensor=o_d.tensor,
                offset=o_d.offset + t * OC,
                ap=[[OPP, OG], [BP * OUT_F, BG], [OUT_F, BP], [1, OC]],
            )
            nc.sync.dma_start(out=dst, in_=outc)


def _build():
    if "nc" in _CACHE:
        return _CACHE["nc"]
    import concourse.bacc as bacc
    import concourse.tile as tile
    from concourse import mybir

    f32 = mybir.dt.float32
    bf16 = mybir.dt.bfloat16
    nc = bacc.Bacc(
        "TRN2",
        target_bir_lowering=False,
        debug=False,
        enable_asserts=True,
        num_devices=N_CORES,
    )
    m_d = nc.dram_tensor("m0", [B_SH, IN_F], f32, kind="ExternalInput").ap()
    w_d = nc.dram_tensor("w0", [IN_F, OUT_F * AGG], f32, kind="ExternalInput").ap()
    i_d = nc.dram_tensor("ident0", [128, 128], bf16, kind="ExternalInput").ap()
    o_d = nc.dram_tensor("out0", [B_SH, OUT_F], f32, kind="ExternalOutput").ap()
    with tile.TileContext(nc) as tc:
        emit_core_program(tc, o_d, m_d, w_d, i_d)
    nc.compile()
    _CACHE["nc"] = nc
    return nc


def run(m, weight, trace=False, **spmd_kwargs):
    """Run on 8 NeuronCores; returns (full_output, BassKernelResults)."""
    from concourse.bass_utils import run_bass_kernel_spmd

    nc = _build()
    m = np.ascontiguousarray(np.asarray(m, dtype=np.float32))
    weight = np.ascontiguousarray(np.asarray(weight, dtype=np.float32))
    assert m.shape == (B, IN_F) and weight.shape == (IN_F, OUT_F * AGG)
    ident = _identity_np()
    in_maps = [
        {"m0": m[i * B_SH : (i + 1) * B_SH], "w0": weight, "ident0": ident}
        for i in range(N_CORES)
    ]
    res = run_bass_kernel_spmd(
        nc, in_maps, core_ids=list(range(N_CORES)), trace=trace, **spmd_kwargs
    )
    out = np.concatenate([res.results[i]["out0"] for i in range(N_CORES)], axis=0)
    return out, res


def kernel(m, weight, agg_features=AGG, **_ignored):
    assert int(agg_features) == AGG
    out, _ = run(m, weight, trace=False)
    return out.astype(np.float32)


# revision 5
# speedup vs baseline: 1.6026x; 1.0065x over previous
"""Trainium2 Bass kernel for nn_MaxMinAgg.

Computes, for full inputs m [1024, 256] f32 and weight [256, 512] f32:
    z[b, j]  = max_k min(m[b, k], weight[k, j])          (tropical max-min matmul)
    out[b,o] = max_a z[b, 4*o + a]                       (max-pool over AGG=4 groups)

Identity: max_a min(x, w_a) = min(x, max_a w_a): the AGG max-pool folds into the
weight (wmax[k, o] = max_a weight[k, 4o+a]), so
    out[b, o] = max_k min(m[b, k], wmax[k, o])

Distribution: data-parallel over batch across 8 NeuronCores (128 rows each);
weight replicated.

Per-core algorithm (v4 — bf16, low-replication layout, max-tree, latency-tuned
DMA schedule):
  - All compute in bf16: min/max are exact selections, so the only error is the
    initial f32->bf16 rounding (<= 2^-9 relative, far under the 2e-2 gate), and
    bf16 unlocks the DVE's 2x packed mode for tensor_tensor (tensor_reduce is
    always 1x, hence the halving max-tree below).
  - Partition layout p = og*16 + bg (OG=8 output groups x BG=16 batch groups).
    Partition p owns the full k-reduction for an 8b' x 16o' output tile, so
    k (=256) lives on the free axis and there is NO cross-partition combine.
  - m path: natural 128KB load, one small ACT convert, 64KB bf16 write-back,
    then a 512KB broadcast read (8KB-contiguous runs) gives m16rep [p, b', k].
  - w path: 512KB load split across both HW queues -> per-half DVE fold + PE
    transpose + ACT copy -> wmaxT [o, k] bf16 -> DRAM -> 512KB of broadcast
    reads give w_rep [p, o', k] (transpose outputs must start at PSUM
    partition 0, so direct placement at partition offsets is impossible).
  - Only the scalar and sync queues have hardware descriptor generators; each
    DMA hop costs ~2us issue->data latency, so the schedule keeps the w chain
    unobstructed on scalar (identity first — it gates the PE transposes),
    splits each w_rep chunk across both queues, and slots the m-side reads
    into the sync queue gaps.
  - Hot loop per o'-half: bf16 tensor_tensor min (m16rep broadcast over o',
    w_rep broadcast over b'; chunk 0's min is split in two b' halves so it
    starts before the second m16rep chunk lands), a halving tensor_tensor max
    tree over k (256->...->16, all 2x, contiguous halves so packing holds),
    then one 1x tensor_reduce of the last 16 straight to f32 and a
    scatter-DMA to the natural [b, o] layout.
"""

import sys

import numpy as np

if "/opt/trn_rl_repo" not in sys.path:
    sys.path.insert(0, "/opt/trn_rl_repo")

B, IN_F, OUT_F, AGG = 1024, 256, 128, 4
N_CORES = 8
B_SH = B // N_CORES  # 128

OG, BG = 8, 16  # partition p = og*BG + bg
BP = B_SH // BG  # 8 batch rows per partition
OPP = OUT_F // OG  # 16 output cols per partition
NT = 2  # o'-halves (chunks of the hot loop)
OC = OPP // NT  # 8 output cols per chunk

_CACHE = {}


def _identity_np():
    import ml_dtypes

    return np.eye(128, dtype=ml_dtypes.bfloat16)


def emit_core_program(tc, o_d, m_d, w_d, i_d):
    """Emit the per-core Tile program.

    o_d: DRAM out [B_SH, OUT_F] f32, m_d: DRAM in [B_SH, IN_F] f32,
    w_d: DRAM in [IN_F, OUT_F*AGG] f32, i_d: DRAM in [128, 128] bf16 identity.
    """
    from contextlib import ExitStack

    import concourse.bass as bass
    from concourse import mybir

    nc = tc.nc
    f32 = mybir.dt.float32
    bf16 = mybir.dt.bfloat16
    AX = mybir.AxisListType
    OP = mybir.AluOpType

    with ExitStack() as ctx:
        const = ctx.enter_context(tc.tile_pool(name="const", bufs=1))
        mintp = ctx.enter_context(tc.tile_pool(name="mintp", bufs=2))
        treep = ctx.enter_context(tc.tile_pool(name="treep", bufs=2))
        ps_tr = ctx.enter_context(tc.tile_pool(name="ps_tr", bufs=2, space="PSUM"))

        # --- scalar queue head: identity (gates the PE transposes), the m
        # natural load, then the w k-half this queue owns.
        ident = const.tile([128, 128], bf16)
        nc.scalar.dma_start(out=ident, in_=i_d)
        m32n = const.tile([128, IN_F], f32)
        nc.scalar.dma_start(out=m32n, in_=m_d)
        w_sb = const.tile([128, 2, OUT_F * AGG], f32)
        wv = w_d.rearrange("(h p) j -> p h j", p=128)
        nc.scalar.dma_start(out=w_sb[:, 0, :], in_=wv[:, 0, :])
        # sync queue head: the other w k-half.
        nc.sync.dma_start(out=w_sb[:, 1, :], in_=wv[:, 1, :])

        # m -> bf16 (one tiny ACT op) -> DRAM, so the broadcast read moves
        # 512KB instead of 1MB.
        m16n = const.tile([128, IN_F], bf16)
        nc.scalar.copy(out=m16n, in_=m32n)
        m16_d = nc.dram_tensor("m16_scratch", [B_SH, IN_F], bf16, kind="Internal").ap()
        nc.scalar.dma_start(out=m16_d, in_=m16n)

        # m16rep[p = og*BG+bg, b', k] = m16[bg*BP + b', k], two b'-half chunks
        # on the sync queue (chunk 0 unblocks the first min early).
        m16rep = const.tile([128, BP, IN_F], bf16)
        HB = BP * IN_F // 2
        for c in range(2):
            src = bass.AP(
                tensor=m16_d.tensor,
                offset=m16_d.offset + c * HB,
                ap=[[0, OG], [BP * IN_F, BG], [1, HB]],
            )
            nc.sync.dma_start(
                out=m16rep[:, c * (BP // 2) : (c + 1) * (BP // 2), :], in_=src
            )

        # --- weight fold + transpose, pipelined per k-half h (h=1 lands on
        # the sync queue first). wmax[k_p, h, o] = max_a w[k, 4o+a] -> bf16,
        # PE transpose, ACT copy PSUM -> wmaxT16 [o_p, h, k'] == wmaxT [o, k].
        wmax16 = const.tile([128, 2, OUT_F], bf16)
        wmaxT16 = const.tile([128, 2, 128], bf16)
        for h in (1, 0):
            nc.vector.tensor_reduce(
                out=wmax16[:, h, :],
                in_=w_sb.rearrange("p h (o a) -> p h o a", a=AGG)[:, h, :, :],
                axis=AX.X,
                op=OP.max,
            )
            pt = ps_tr.tile([128, 128], bf16, tag="ptr")
            nc.tensor.transpose(pt, wmax16[:, h, :], ident)
            nc.scalar.copy(out=wmaxT16[:, h, :], in_=pt)

        wT_d = nc.dram_tensor("wT_scratch", [OUT_F, IN_F], bf16, kind="Internal").ap()
        nc.scalar.dma_start(out=wT_d, in_=wmaxT16)

        # w_rep[t][p = og*BG+bg, o', k] = wmaxT[og*OPP + t*OC + o', k], each
        # chunk split og-wise across the two HW queues.
        wreps = []
        for t in range(NT):
            wr = const.tile([128, OC, IN_F], bf16, name=f"wrep{t}")
            for half, q in ((0, nc.scalar), (1, nc.sync)):
                src = bass.AP(
                    tensor=wT_d.tensor,
                    offset=wT_d.offset + (half * (OG // 2) * OPP + t * OC) * IN_F,
                    ap=[[OPP * IN_F, OG // 2], [0, BG], [1, OC * IN_F]],
                )
                q.dma_start(out=wr[half * 64 : (half + 1) * 64, :, :], in_=src)
            wreps.append(wr)

        m16v = m16rep.rearrange("p b k -> p b () k")

        # --- hot loop: per o'-half, bf16 min + halving max-tree over k ------
        for t in range(NT):
            mint = mintp.tile([128, BP, OC, IN_F], bf16, tag="mint")
            # Chunk 0's min is split in two b' halves: the first half only
            # needs the first m16rep chunk, so it starts as soon as w_rep0
            # lands.
            bsplits = (2, 1)[t]
            bs = BP // bsplits
            for s in range(bsplits):
                nc.vector.tensor_tensor(
                    out=mint[:, s * bs : (s + 1) * bs, :, :],
                    in0=m16v[:, s * bs : (s + 1) * bs, :, :].broadcast_to(
                        (128, bs, OC, IN_F)
                    ),
                    in1=wreps[t]
                    .rearrange("p o k -> p () o k")
                    .broadcast_to((128, bs, OC, IN_F)),
                    op=OP.min,
                )
            cur, n = mint, IN_F
            while n > 16:
                h = n // 2
                nxt = treep.tile([128, BP, OC, h], bf16, tag=f"tree{h}")
                nc.vector.tensor_tensor(
                    out=nxt,
                    in0=cur[:, :, :, :h],
                    in1=cur[:, :, :, h:],
                    op=OP.max,
                )
                cur, n = nxt, h
            part = treep.tile([128, BP, OC], f32, tag="part")
            nc.vector.tensor_reduce(out=part, in_=cur, axis=AX.X, op=OP.max)
            dst = bass.AP(
                tensor=o_d.tensor,
                offset=o_d.offset + t * OC,
                ap=[[OPP, OG], [BP * OUT_F, BG], [OUT_F, BP], [1, OC]],
            )
            nc.sync.dma_start(out=dst, in_=part)


def _build():
    if "nc" in _CACHE:
        return _CACHE["nc"]
    import concourse.bacc as bacc
    import concourse.tile as tile
    from concourse import mybir

    f32 = mybir.dt.float32
    bf16 = mybir.dt.bfloat16
    nc = bacc.Bacc(
        "TRN2",
        target_bir_lowering=False,
        debug=False,
        enable_asserts=True,
        num_devices=N_CORES,
    )
    m_d = nc.dram_tensor("m0", [B_SH, IN_F], f32, kind="ExternalInput").ap()
    w_d = nc.dram_tensor("w0", [IN_F, OUT_F * AGG], f32, kind="ExternalInput").ap()
    i_d = nc.dram_tensor("ident0", [128, 128], bf16, kind="ExternalInput").ap()
    o_d = nc.dram_tensor("out0", [B_SH, OUT_F], f32, kind="ExternalOutput").ap()
    with tile.TileContext(nc) as tc:
        emit_core_program(tc, o_d, m_d, w_d, i_d)
    nc.compile()
    _CACHE["nc"] = nc
    return nc


def run(m, weight, trace=False, **spmd_kwargs):
    """Run on 8 NeuronCores; returns (full_output, BassKernelResults)."""
    from concourse.bass_utils import run_bass_kernel_spmd

    nc = _build()
    m = np.ascontiguousarray(np.asarray(m, dtype=np.float32))
    weight = np.ascontiguousarray(np.asarray(weight, dtype=np.float32))
    assert m.shape == (B, IN_F) and weight.shape == (IN_F, OUT_F * AGG)
    ident = _identity_np()
    in_maps = [
        {"m0": m[i * B_SH : (i + 1) * B_SH], "w0": weight, "ident0": ident}
        for i in range(N_CORES)
    ]
    res = run_bass_kernel_spmd(
        nc, in_maps, core_ids=list(range(N_CORES)), trace=trace, **spmd_kwargs
    )
    out = np.concatenate([res.results[i]["out0"] for i in range(N_CORES)], axis=0)
    return out, res


def kernel(m, weight, agg_features=AGG, **_ignored):
    assert int(agg_features) == AGG
    out, _ = run(m, weight, trace=False)
    return out.astype(np.float32)
